# revision 27
# baseline (speedup 1.0000x reference)
"""DeepISTA (100 unrolled FISTA iterations) on 8 TRN2 NeuronCores.

Problem: y (256, 16384) f32, D (256, 512) f32 ->
         out = z_100 (512, 16384) f32 from

    out_k = softshrink(z_{k-1} - step*D^T(D z_{k-1} - y), thr)
    z_k   = out_k + mu_k (out_k - out_{k-1})

Sharding: data-parallel over samples (16384 -> 8 x 2048); D replicated.
No inter-core communication (pure forward iteration).

PE-roofline formulation (f32r matmuls, 1 cyc/row): the only PE work per
iteration is the two dense matmul passes,

    phase A:  psum_r = D @ Z            (32 MMs: m2 x n4 x kx4, [128,512])
              r = gamma_{k-1} y + psum  (ACT copy + Pool add vs scaled y)
    phase B:  psum_u = (step*D)^T @ r   (32 MMs over x-chunks and n-cols)
              O'_k = FSHRINK(psum_u, Z) (DVE custom op; immediates absorb
                                         the gamma rescale and c_k, thr)
              Z    = O'_k - O'_{k-1}    (plain Pool subtract)

64 matmuls x 512 cols = 32768 PE cycles = 13.65 us/iter @ 2.4 GHz, and the
schedule hits that roofline in the cost model: the gamma-rescaled state
(Z_k = -gamma_k z_k, gamma_{k-1} = gamma_k beta_k, gamma_{N-1} = -1) turns
the FISTA momentum into a scalar-free subtract, so the DVE queue carries
ONLY the 8 FSHRINKs — the critical latency chain FSHRINK -> momentum ->
next MM1 loses the ACT hop and lands ~1.4us early.  The y-combine runs on
ACT+Pool against a per-iteration scaled-y tensor (2 ACT copies, no data
deps), the z state is stored f32r (BIR verifier requirement for f32r
matmul operands), and k=0 degenerates to phase B on y with the scale
folded into FSHRINK's C0.  PSUM: pr 2x[128,512] + pu 3x[128,1024]; the pu
triple-buffer absorbs the MM2-vs-FSHRINK rate mismatch inside phase B.
"""

import sys

if "/opt/trn_rl_repo" not in sys.path:
    sys.path.insert(0, "/opt/trn_rl_repo")

import numpy as np

# ---------------------------------------------------------------- constants
N_ITER = 100
LAMBD = 0.1
LIPSCHITZ = 8.0
DIM_Y, DIM_X, N_SAMPLES = 256, 512, 16384
N_CORES = 8
NSH = N_SAMPLES // N_CORES  # 2048 samples per core

# phase-B elementwise granularity in columns (512 or 1024).  NOTE: 512
# looks better in CoreSim (hits the 13.65us/iter PE roofline exactly) but
# is ~2.5x SLOWER on real hardware — the 48 small vector ops per
# iteration trigger per-op/semaphore overheads the simulator does not
# model.  1024 is the hardware-validated setting.
PB_GRAN = 1024
# engine for the psum_r - y combine, per chunk:
#   "v"  = DVE scalar_tensor_tensor in one op
#   "ap" = ACT copy psum->r, then Pool combine with y in SBUF
# All-"ap" keeps the DVE queue pure-FSHRINK, which is what lets the PE
# stay at its roofline (DVE is the critical latency chain into the next
# iteration's MM1).
YSUB_ENG = ("ap",) * 8
# momentum, per phase-B chunk: "v" = DVE stt, else Pool tensor ops
MOM_ENG = ("gp",) * 8
# psum pool buffer counts: pr 2 x [128,512] (2 banks) + pu 3 x [128,1024]
# (6 banks) = 8 banks; pu triple-buffering absorbs the MM2(0.85us) vs
# FSHRINK(1.19us) rate mismatch inside phase B
PR_BUFS = 2
PU_BUFS = 3
# gamma-rescaled formulation: state Z_k = -gamma_k z_k with
# gamma_{k-1} = gamma_k * beta_k, gamma_{N-1} = -1.  FSHRINK's immediates
# absorb the rescale (C0 = -c_k gamma_k/gamma_{k-1}, clip = |gamma_k| c_k thr)
# and the momentum collapses to a plain subtract Z_k = O'_k - O'_{k-1}
# (no ACT scale op).  The y-combine becomes r = gamma_{k-1} y + psum, done
# either as a DVE stt or as ACT-copy + Pool-add against a per-iteration
# scaled-y tensor (updated by the otherwise-idle ACT engine).  Validated
# numerically: ~1.5e-3 rel err, fp32 range safe (|gamma_0| ~ 6e-33).
GAMMA = True
# ysub granularity in columns under GAMMA (512 -> 8 ops on [128,512]
# psums, 1024 -> 4 ops on [128,1024] psums; PR_BUFS tiles of that width)
YSUB_GRAN = 512

_BUILD_CACHE = {}


# ------------------------------------------------------- custom DVE op
def _get_fshrink():
    import concourse.dve_ops as dve_ops_mod
    from concourse.dve_ops import DveOp
    from concourse.dve_spec import Spec, Src0, Src1, C0, C1, C2, maxx, minn, lower
    from concourse.dve_uop import DveOpSpec

    if any(op.name == "FSHRINK_ANT" for op in dve_ops_mod.OPS):
        return next(op for op in dve_ops_mod.OPS if op.name == "FSHRINK_ANT")

    def _ref(in0, in1, s0, s1, imm2):
        x = (s0 * (in0 - in1)).astype(np.float32)
        return (x - np.clip(x, s1, imm2)).astype(np.float32)

    x = (Src0 - Src1) * C0
    body = x - minn(maxx(x, C1), C2)
    spec = Spec(body=body, reference=_ref)

    row = max(dve_ops_mod._SUB_OPCODE_FOR_NAME.values()) + 1
    assert row < 0x20
    shas = {}
    for ver in ("v3",):
        uops = lower(spec, ver=ver)
        shas[ver] = DveOpSpec(
            name="FSHRINK_ANT", opcode=row, uops=uops, rd1_en=True
        ).sha(ver)
    op = DveOp("FSHRINK_ANT", spec, subdim=False, uops_sha=shas)
    dve_ops_mod.OPS.append(op)
    dve_ops_mod.CUSTOM_DVE_SPECS[op.name] = op.spec
    dve_ops_mod._SUB_OPCODE_FOR_NAME[op.name] = row
    return op


# ------------------------------------------------------- iteration scalars
def _fista_scalars(n_iter):
    """Replicate the reference's fp32 t-recurrence exactly."""
    f32 = np.float32
    t_old = f32(1.0)
    mus = []
    for _ in range(n_iter):
        t = f32(0.5) * (f32(1.0) + np.sqrt(f32(1.0) + f32(4.0) * t_old * t_old))
        mus.append((t_old - f32(1.0)) / t)
        t_old = t
    cs = [f32(1.0) + m for m in mus]  # 1+mu_k
    betas = [f32(0.0)] + [mus[k] / cs[k - 1] for k in range(1, n_iter)]
    step = f32(1.0) / f32(LIPSCHITZ)
    thr = step * f32(LAMBD)
    cthrs = [c * thr for c in cs]
    return cs, betas, cthrs


# ------------------------------------------------------- bass module build
def _build(n_iter=N_ITER):
    key = (n_iter, PB_GRAN, YSUB_ENG, MOM_ENG, PR_BUFS, PU_BUFS, GAMMA, YSUB_GRAN)
    if key in _BUILD_CACHE:
        return _BUILD_CACHE[key]

    from concourse import bacc
    import concourse.mybir as mybir
    import concourse.tile as tile

    FSHRINK = _get_fshrink()
    F32 = mybir.dt.float32
    F32R = mybir.dt.float32r
    ALU = mybir.AluOpType
    ACTF = mybir.ActivationFunctionType

    cs, betas, cthrs = _fista_scalars(n_iter)
    n_pb = NSH // PB_GRAN  # phase-B column blocks (4 at 512, 2 at 1024)
    # backward gamma recursion (fp32, matching the numpy validation)
    gam = [np.float32(0.0)] * n_iter
    gam[n_iter - 1] = np.float32(-1.0)
    for kk in range(n_iter - 1, 0, -1):
        gam[kk - 1] = np.float32(gam[kk] * betas[kk])

    nc = bacc.Bacc()
    y_d = nc.dram_tensor("y", [DIM_Y, NSH], F32R, kind="ExternalInput")
    dt_d = nc.dram_tensor("dt", [DIM_X, DIM_Y], F32R, kind="ExternalInput")  # D^T
    sd_d = nc.dram_tensor("sd", [DIM_Y, DIM_X], F32R, kind="ExternalInput")  # step*D
    out_d = nc.dram_tensor("out", [DIM_X, NSH], F32, kind="ExternalOutput")

    with tile.TileContext(nc) as tc:
        with (
            tc.tile_pool(name="sb", bufs=1) as sb,
            tc.tile_pool(name="pr", bufs=PR_BUFS, space="PSUM") as prp,
            tc.tile_pool(name="pu", bufs=PU_BUFS, space="PSUM") as pup,
        ):
            y2 = [sb.tile([128, NSH], F32R, tag=f"y{m}", name=f"y{m}") for m in range(2)]
            r2 = [sb.tile([128, NSH], F32R, tag=f"r{m}", name=f"r{m}") for m in range(2)]
            # gamma mode with "ap" ysubs: per-iteration scaled y (updated by
            # the otherwise-idle ACT engine; no data deps, issued early)
            need_ys = GAMMA and any(e == "ap" for e in YSUB_ENG)
            if need_ys:
                ys2 = [
                    sb.tile([128, NSH], F32, tag=f"ys{m}", name=f"ys{m}")
                    for m in range(2)
                ]
            # z state, stored f32r (the engines round on write so the BIR
            # verifier accepts it as an f32r matmul operand); elementwise
            # consumers read it bitcast as f32
            z32 = [sb.tile([128, NSH], F32R, tag=f"z{i}", name=f"z{i}") for i in range(4)]
            Oa = [sb.tile([128, NSH], F32, tag=f"Oa{i}", name=f"Oa{i}") for i in range(4)]
            Ob = [sb.tile([128, NSH], F32, tag=f"Ob{i}", name=f"Ob{i}") for i in range(4)]
            # D^T packed [128, 4*256]: chunk kx at cols kx*256:(kx+1)*256
            DTt = sb.tile([128, 4 * DIM_Y], F32R, tag="DTt", name="DTt")
            # step*D packed [128, 2*512]: chunk kc at cols kc*512:(kc+1)*512
            sDt = sb.tile([128, 2 * DIM_X], F32R, tag="sDt", name="sDt")

            # initial loads: k=0 needs sDt + y first; DTt needed from k=1
            for i in range(2):
                nc.scalar.dma_start(
                    sDt[:, i * DIM_X : (i + 1) * DIM_X],
                    sd_d[i * 128 : (i + 1) * 128, :],
                )
            # y in column blocks so k=0's first matmuls start early
            for nb in range(4):
                csl = slice(nb * 512, (nb + 1) * 512)
                nc.gpsimd.dma_start(y2[0][:, csl], y_d[0:128, csl])
                nc.sync.dma_start(y2[1][:, csl], y_d[128:256, csl])
            for i in range(4):
                # DTt is not needed until k=1's phase A; queue it behind y2[1]
                nc.sync.dma_start(
                    DTt[:, i * DIM_Y : (i + 1) * DIM_Y],
                    dt_d[i * 128 : (i + 1) * 128, :],
                )

            # O_old zeros for k=0 (read by FSHRINK in1 and the k=0 momentum).
            # DVE/Pool alternation measured best (all-Pool serializes ahead
            # of the k=0 momentum subs; ACT has no memset).
            for i in range(4):
                (nc.gpsimd if i % 2 else nc.vector).memset(Oa[i][:], 0.0)

            for k in range(n_iter):
                c_k = float(cs[k])
                beta_k = float(betas[k])
                cthr_k = float(cthrs[k])
                O_old = Oa if k % 2 == 0 else Ob
                O_new = Ob if k % 2 == 0 else Oa

                # ---- phase A: psum_r = D @ z; r = psum_r - y (skip at k=0)
                # (gamma mode: r = gamma_{k-1} y + psum_r via DVE stt)
                if k > 0 and GAMMA:
                    g_km1 = float(gam[k - 1])
                    if need_ys:
                        # ys = gamma_{k-1} * y (ACT, no deps on this iter)
                        for m in range(2):
                            nc.scalar.activation(
                                ys2[m][:],
                                y2[m][:].bitcast(F32),
                                ACTF.Copy,
                                bias=0.0,
                                scale=g_km1,
                            )
                    npr = YSUB_GRAN // 512
                    for nb in range(4 // npr):
                        bsl = slice(nb * YSUB_GRAN, (nb + 1) * YSUB_GRAN)
                        for m in range(2):
                            pr = prp.tile([128, YSUB_GRAN], F32, tag="pr", name="pr")
                            for ns in range(npr):
                                n = nb * npr + ns
                                nsl = slice(n * 512, (n + 1) * 512)
                                psl = slice(ns * 512, (ns + 1) * 512)
                                for kx in range(4):
                                    nc.tensor.matmul(
                                        pr[:, psl],
                                        DTt[:, kx * 256 + m * 128 : kx * 256 + (m + 1) * 128],
                                        z32[kx][:, nsl],
                                        start=(kx == 0),
                                        stop=(kx == 3),
                                    )
                            if YSUB_ENG[nb * 2 + m] == "v":
                                nc.vector.scalar_tensor_tensor(
                                    r2[m][:, bsl],
                                    y2[m][:, bsl].bitcast(F32),
                                    g_km1,
                                    pr[:],
                                    op0=ALU.mult,
                                    op1=ALU.add,
                                )
                            else:
                                # r = psum (ACT), then r += ys (Pool, SBUF)
                                nc.scalar.activation(
                                    r2[m][:, bsl], pr[:], ACTF.Copy
                                )
                                nc.gpsimd.tensor_tensor(
                                    r2[m][:, bsl],
                                    r2[m][:, bsl].bitcast(F32),
                                    ys2[m][:, bsl],
                                    op=ALU.add,
                                )
                elif k > 0:
                    for n in range(4):
                        nsl = slice(n * 512, (n + 1) * 512)
                        for m in range(2):
                            pr = prp.tile([128, 512], F32, tag="pr", name="pr")
                            for kx in range(4):
                                nc.tensor.matmul(
                                    pr[:],
                                    DTt[:, kx * 256 + m * 128 : kx * 256 + (m + 1) * 128],
                                    z32[kx][:, nsl],
                                    start=(kx == 0),
                                    stop=(kx == 3),
                                )
                            if YSUB_ENG[n * 2 + m] == "v":
                                nc.vector.scalar_tensor_tensor(
                                    r2[m][:, nsl],
                                    y2[m][:, nsl].bitcast(F32),
                                    -1.0,
                                    pr[:],
                                    op0=ALU.mult,
                                    op1=ALU.add,
                                )
                            else:
                                nc.scalar.activation(
                                    r2[m][:, nsl], pr[:], ACTF.Copy
                                )
                                nc.gpsimd.tensor_tensor(
                                    r2[m][:, nsl],
                                    r2[m][:, nsl].bitcast(F32),
                                    y2[m][:, nsl].bitcast(F32),
                                    op=ALU.subtract,
                                )

                # ---- phase B: psum_u = (step*D)^T @ (r | y); shrink; momentum
                for nb in range(n_pb):
                    hsl = slice(nb * PB_GRAN, (nb + 1) * PB_GRAN)
                    for mx in range(4):
                        ci = nb * 4 + mx
                        pu = pup.tile([128, PB_GRAN], F32, tag="pu", name="pu")
                        for ns in range(PB_GRAN // 512):
                            nsl = slice(
                                nb * PB_GRAN + ns * 512,
                                nb * PB_GRAN + (ns + 1) * 512,
                            )
                            psl = slice(ns * 512, (ns + 1) * 512)
                            mov = y2 if k == 0 else r2
                            for kc in range(2):
                                nc.tensor.matmul(
                                    pu[:, psl],
                                    sDt[:, kc * 512 + mx * 128 : kc * 512 + (mx + 1) * 128],
                                    mov[kc][:, nsl],
                                    start=(kc == 0),
                                    stop=(kc == 1),
                                )
                        if GAMMA:
                            # O'_k = gamma_k O_k: C0 and the clip absorb the
                            # rescale (k=0: psum = +b, in1 = zeros)
                            if k == 0:
                                s0 = float(-gam[0] * cs[0])
                                cl = float(abs(gam[0]) * cthrs[0])
                            else:
                                s0 = float(-cs[k] * gam[k] / gam[k - 1])
                                cl = float(abs(gam[k]) * cthrs[k])
                        else:
                            s0 = c_k if k > 0 else -c_k
                            cl = cthr_k
                        nc.vector._custom_dve(
                            FSHRINK,
                            out=O_new[mx][:, hsl],
                            in0=pu[:],
                            in1=(
                                z32[mx][:, hsl].bitcast(F32)
                                if k > 0
                                else O_old[mx][:, hsl]
                            ),
                            s0=s0,
                            s1=-cl,
                            imm2=cl,
                        )
                        # momentum
                        if GAMMA:
                            # Z_k = O'_k - O'_{k-1} (plain subtract; O_old is
                            # zeros at k=0 so Z_0 = O'_0)
                            if MOM_ENG[ci] == "v" and k > 0:
                                nc.vector.scalar_tensor_tensor(
                                    z32[mx][:, hsl],
                                    O_old[mx][:, hsl],
                                    -1.0,
                                    O_new[mx][:, hsl],
                                    op0=ALU.mult,
                                    op1=ALU.add,
                                )
                            else:
                                nc.gpsimd.tensor_tensor(
                                    z32[mx][:, hsl],
                                    O_new[mx][:, hsl],
                                    O_old[mx][:, hsl],
                                    op=ALU.subtract,
                                )
                        elif k == 0:
                            # beta_0 = 0 and O_old is zeros: z = -O_new
                            nc.gpsimd.tensor_tensor(
                                z32[mx][:, hsl],
                                O_old[mx][:, hsl],
                                O_new[mx][:, hsl],
                                op=ALU.subtract,
                            )
                        elif MOM_ENG[ci] == "v":
                            nc.vector.scalar_tensor_tensor(
                                z32[mx][:, hsl],
                                O_old[mx][:, hsl],
                                beta_k,
                                O_new[mx][:, hsl],
                                op0=ALU.mult,
                                op1=ALU.subtract,
                            )
                        else:
                            if MOM_ENG[ci] == "ap":
                                nc.scalar.activation(
                                    O_old[mx][:, hsl],
                                    O_old[mx][:, hsl],
                                    ACTF.Copy,
                                    bias=0.0,
                                    scale=beta_k,
                                )
                            else:  # "gp"
                                nc.gpsimd.tensor_scalar_mul(
                                    O_old[mx][:, hsl], O_old[mx][:, hsl], beta_k
                                )
                            nc.gpsimd.tensor_tensor(
                                z32[mx][:, hsl],
                                O_old[mx][:, hsl],
                                O_new[mx][:, hsl],
                                op=ALU.subtract,
                            )
                        if k == n_iter - 1:
                            # stream the output as each chunk finalizes
                            eng = (nc.sync, nc.scalar, nc.gpsimd, nc.sync)[mx]
                            eng.dma_start(
                                out_d[mx * 128 : (mx + 1) * 128, hsl],
                                z32[mx][:, hsl].bitcast(F32),
                            )

    nc.compile()
    _BUILD_CACHE[key] = nc
    return nc


# ------------------------------------------------------- host-side driver
def _host_inputs(y, D):
    DT = np.ascontiguousarray(D.T.astype(np.float32))
    sD = np.ascontiguousarray((np.float32(1.0 / LIPSCHITZ) * D).astype(np.float32))
    in_maps = []
    for c in range(N_CORES):
        ysh = np.ascontiguousarray(y[:, c * NSH : (c + 1) * NSH].astype(np.float32))
        in_maps.append({"y": ysh, "dt": DT, "sd": sD})
    return in_maps


LAST_EXEC_NS = None


def kernel(y, D):
    global LAST_EXEC_NS
    import os

    from concourse.bass_utils import run_bass_kernel_spmd

    y = np.asarray(y, dtype=np.float32)
    D = np.asarray(D, dtype=np.float32)
    assert y.shape == (DIM_Y, N_SAMPLES) and D.shape == (DIM_Y, DIM_X)

    nc = _build(N_ITER)
    in_maps = _host_inputs(y, D)
    trace = os.environ.get("DEEPISTA_TRACE", "0") == "1"
    r = run_bass_kernel_spmd(nc, in_maps, list(range(N_CORES)), trace=trace)
    LAST_EXEC_NS = r.exec_time_ns
    out = np.concatenate([r.results[c]["out"] for c in range(N_CORES)], axis=1)
    return out.astype(np.float32)


# revision 30
# speedup vs baseline: 1.0005x; 1.0005x over previous
"""DeepISTA (100 unrolled FISTA iterations) on 8 TRN2 NeuronCores.

Problem: y (256, 16384) f32, D (256, 512) f32 ->
         out = z_100 (512, 16384) f32 from

    out_k = softshrink(z_{k-1} - step*D^T(D z_{k-1} - y), thr)
    z_k   = out_k + mu_k (out_k - out_{k-1})

Sharding: data-parallel over samples (16384 -> 8 x 2048); D replicated.
No inter-core communication (pure forward iteration).

PE-roofline formulation (f32r matmuls, 1 cyc/row): the only PE work per
iteration is the two dense matmul passes,

    phase A:  psum_r = D @ Z            (32 MMs: m2 x n4 x kx4, [128,512])
              r = gamma_{k-1} y + psum  (ACT copy + Pool add vs scaled y)
    phase B:  psum_u = (step*D)^T @ r   (32 MMs over x-chunks and n-cols)
              O'_k = FSHRINK(psum_u, Z) (DVE custom op; immediates absorb
                                         the gamma rescale and c_k, thr)
              Z    = O'_k - O'_{k-1}    (plain Pool subtract)

64 matmuls x 512 cols = 32768 PE cycles = 13.65 us/iter @ 2.4 GHz, and the
schedule hits that roofline in the cost model: the gamma-rescaled state
(Z_k = -gamma_k z_k, gamma_{k-1} = gamma_k beta_k, gamma_{N-1} = -1) turns
the FISTA momentum into a scalar-free subtract, so the DVE queue carries
ONLY the 8 FSHRINKs — the critical latency chain FSHRINK -> momentum ->
next MM1 loses the ACT hop and lands ~1.4us early.  The y-combine runs on
ACT+Pool against a per-iteration scaled-y tensor (2 ACT copies, no data
deps), the z state is stored f32r (BIR verifier requirement for f32r
matmul operands), and k=0 degenerates to phase B on y with the scale
folded into FSHRINK's C0.  PSUM: pr 2x[128,512] + pu 3x[128,1024]; the pu
triple-buffer absorbs the MM2-vs-FSHRINK rate mismatch inside phase B.
"""

import sys

if "/opt/trn_rl_repo" not in sys.path:
    sys.path.insert(0, "/opt/trn_rl_repo")

import numpy as np

# ---------------------------------------------------------------- constants
N_ITER = 100
LAMBD = 0.1
LIPSCHITZ = 8.0
DIM_Y, DIM_X, N_SAMPLES = 256, 512, 16384
N_CORES = 8
NSH = N_SAMPLES // N_CORES  # 2048 samples per core

# phase-B elementwise granularity in columns (512 or 1024).  NOTE: 512
# looks better in CoreSim (hits the 13.65us/iter PE roofline exactly) but
# is ~2.5x SLOWER on real hardware — the 48 small vector ops per
# iteration trigger per-op/semaphore overheads the simulator does not
# model.  1024 is the hardware-validated setting.
PB_GRAN = 1024
# engine for the psum_r - y combine, per chunk:
#   "v"  = DVE scalar_tensor_tensor in one op
#   "ap" = ACT copy psum->r, then Pool combine with y in SBUF
# All-"ap" keeps the DVE queue pure-FSHRINK, which is what lets the PE
# stay at its roofline (DVE is the critical latency chain into the next
# iteration's MM1).
YSUB_ENG = ("ap",) * 8
# momentum, per phase-B chunk: "v" = DVE stt, else Pool tensor ops
MOM_ENG = ("gp",) * 8
# psum pool buffer counts: pr 2 x [128,512] (2 banks) + pu 3 x [128,1024]
# (6 banks) = 8 banks; pu triple-buffering absorbs the MM2(0.85us) vs
# FSHRINK(1.19us) rate mismatch inside phase B
PR_BUFS = 2
PU_BUFS = 3
# gamma-rescaled formulation: state Z_k = -gamma_k z_k with
# gamma_{k-1} = gamma_k * beta_k, gamma_{N-1} = -1.  FSHRINK's immediates
# absorb the rescale (C0 = -c_k gamma_k/gamma_{k-1}, clip = |gamma_k| c_k thr)
# and the momentum collapses to a plain subtract Z_k = O'_k - O'_{k-1}
# (no ACT scale op).  The y-combine becomes r = gamma_{k-1} y + psum, done
# either as a DVE stt or as ACT-copy + Pool-add against a per-iteration
# scaled-y tensor (updated by the otherwise-idle ACT engine).  Validated
# numerically: ~1.5e-3 rel err, fp32 range safe (|gamma_0| ~ 6e-33).
GAMMA = True
# ysub granularity in columns under GAMMA (512 -> 8 ops on [128,512]
# psums, 1024 -> 4 ops on [128,1024] psums; PR_BUFS tiles of that width)
YSUB_GRAN = 512

_BUILD_CACHE = {}


# ------------------------------------------------------- custom DVE op
def _get_fshrink():
    import concourse.dve_ops as dve_ops_mod
    from concourse.dve_ops import DveOp
    from concourse.dve_spec import Spec, Src0, Src1, C0, C1, C2, maxx, minn, lower
    from concourse.dve_uop import DveOpSpec

    if any(op.name == "FSHRINK_ANT" for op in dve_ops_mod.OPS):
        return next(op for op in dve_ops_mod.OPS if op.name == "FSHRINK_ANT")

    def _ref(in0, in1, s0, s1, imm2):
        x = (s0 * (in0 - in1)).astype(np.float32)
        return (x - np.clip(x, s1, imm2)).astype(np.float32)

    x = (Src0 - Src1) * C0
    body = x - minn(maxx(x, C1), C2)
    spec = Spec(body=body, reference=_ref)

    row = max(dve_ops_mod._SUB_OPCODE_FOR_NAME.values()) + 1
    assert row < 0x20
    shas = {}
    for ver in ("v3",):
        uops = lower(spec, ver=ver)
        shas[ver] = DveOpSpec(
            name="FSHRINK_ANT", opcode=row, uops=uops, rd1_en=True
        ).sha(ver)
    op = DveOp("FSHRINK_ANT", spec, subdim=False, uops_sha=shas)
    dve_ops_mod.OPS.append(op)
    dve_ops_mod.CUSTOM_DVE_SPECS[op.name] = op.spec
    dve_ops_mod._SUB_OPCODE_FOR_NAME[op.name] = row
    return op


# ------------------------------------------------------- iteration scalars
def _fista_scalars(n_iter):
    """Replicate the reference's fp32 t-recurrence exactly."""
    f32 = np.float32
    t_old = f32(1.0)
    mus = []
    for _ in range(n_iter):
        t = f32(0.5) * (f32(1.0) + np.sqrt(f32(1.0) + f32(4.0) * t_old * t_old))
        mus.append((t_old - f32(1.0)) / t)
        t_old = t
    cs = [f32(1.0) + m for m in mus]  # 1+mu_k
    betas = [f32(0.0)] + [mus[k] / cs[k - 1] for k in range(1, n_iter)]
    step = f32(1.0) / f32(LIPSCHITZ)
    thr = step * f32(LAMBD)
    cthrs = [c * thr for c in cs]
    return cs, betas, cthrs


# ------------------------------------------------------- bass module build
def _build(n_iter=N_ITER):
    key = (n_iter, PB_GRAN, YSUB_ENG, MOM_ENG, PR_BUFS, PU_BUFS, GAMMA, YSUB_GRAN)
    if key in _BUILD_CACHE:
        return _BUILD_CACHE[key]

    from concourse import bacc
    import concourse.mybir as mybir
    import concourse.tile as tile

    FSHRINK = _get_fshrink()
    F32 = mybir.dt.float32
    F32R = mybir.dt.float32r
    ALU = mybir.AluOpType
    ACTF = mybir.ActivationFunctionType

    cs, betas, cthrs = _fista_scalars(n_iter)
    n_pb = NSH // PB_GRAN  # phase-B column blocks (4 at 512, 2 at 1024)
    # backward gamma recursion (fp32, matching the numpy validation)
    gam = [np.float32(0.0)] * n_iter
    gam[n_iter - 1] = np.float32(-1.0)
    for kk in range(n_iter - 1, 0, -1):
        gam[kk - 1] = np.float32(gam[kk] * betas[kk])

    nc = bacc.Bacc()
    y_d = nc.dram_tensor("y", [DIM_Y, NSH], F32R, kind="ExternalInput")
    dt_d = nc.dram_tensor("dt", [DIM_X, DIM_Y], F32R, kind="ExternalInput")  # D^T
    sd_d = nc.dram_tensor("sd", [DIM_Y, DIM_X], F32R, kind="ExternalInput")  # step*D
    out_d = nc.dram_tensor("out", [DIM_X, NSH], F32, kind="ExternalOutput")

    with tile.TileContext(nc) as tc:
        with (
            tc.tile_pool(name="sb", bufs=1) as sb,
            tc.tile_pool(name="pr", bufs=PR_BUFS, space="PSUM") as prp,
            tc.tile_pool(name="pu", bufs=PU_BUFS, space="PSUM") as pup,
        ):
            y2 = [sb.tile([128, NSH], F32R, tag=f"y{m}", name=f"y{m}") for m in range(2)]
            r2 = [sb.tile([128, NSH], F32R, tag=f"r{m}", name=f"r{m}") for m in range(2)]
            # gamma mode with "ap" ysubs: per-iteration scaled y (updated by
            # the otherwise-idle ACT engine; no data deps, issued early)
            need_ys = GAMMA and any(e == "ap" for e in YSUB_ENG)
            if need_ys:
                ys2 = [
                    sb.tile([128, NSH], F32, tag=f"ys{m}", name=f"ys{m}")
                    for m in range(2)
                ]
            # z state, stored f32r (the engines round on write so the BIR
            # verifier accepts it as an f32r matmul operand); elementwise
            # consumers read it bitcast as f32
            z32 = [sb.tile([128, NSH], F32R, tag=f"z{i}", name=f"z{i}") for i in range(4)]
            Oa = [sb.tile([128, NSH], F32, tag=f"Oa{i}", name=f"Oa{i}") for i in range(4)]
            Ob = [sb.tile([128, NSH], F32, tag=f"Ob{i}", name=f"Ob{i}") for i in range(4)]
            # D^T packed [128, 4*256]: chunk kx at cols kx*256:(kx+1)*256
            DTt = sb.tile([128, 4 * DIM_Y], F32R, tag="DTt", name="DTt")
            # step*D packed [128, 2*512]: chunk kc at cols kc*512:(kc+1)*512
            sDt = sb.tile([128, 2 * DIM_X], F32R, tag="sDt", name="sDt")

            # initial loads: k=0 needs sDt + y first; DTt needed from k=1
            for i in range(2):
                nc.scalar.dma_start(
                    sDt[:, i * DIM_X : (i + 1) * DIM_X],
                    sd_d[i * 128 : (i + 1) * 128, :],
                )
            # y in column blocks so k=0's first matmuls start early
            for nb in range(4):
                csl = slice(nb * 512, (nb + 1) * 512)
                nc.gpsimd.dma_start(y2[0][:, csl], y_d[0:128, csl])
                nc.sync.dma_start(y2[1][:, csl], y_d[128:256, csl])
            for i in range(4):
                # DTt is not needed until k=1's phase A; queue it behind y2[1]
                nc.sync.dma_start(
                    DTt[:, i * DIM_Y : (i + 1) * DIM_Y],
                    dt_d[i * 128 : (i + 1) * 128, :],
                )

            # k=0 zeros: under GAMMA all eight k=0 FSHRINKs share ONE
            # [128, PB_GRAN] zero tile as in1 and the k=0 momentum is a
            # plain copy (Z_0 = O'_0), so the Oa tiles are never
            # initialized (FSHRINK at k=1 writes them first) and the DVE
            # queue carries no memsets at the head.
            if GAMMA:
                zb = sb.tile([128, PB_GRAN], F32, tag="zb", name="zb")
                nc.gpsimd.memset(zb[:], 0.0)
            else:
                for i in range(4):
                    (nc.gpsimd if i % 2 else nc.vector).memset(Oa[i][:], 0.0)

            for k in range(n_iter):
                c_k = float(cs[k])
                beta_k = float(betas[k])
                cthr_k = float(cthrs[k])
                O_old = Oa if k % 2 == 0 else Ob
                O_new = Ob if k % 2 == 0 else Oa

                # ---- phase A: psum_r = D @ z; r = psum_r - y (skip at k=0)
                # (gamma mode: r = gamma_{k-1} y + psum_r via DVE stt)
                if k > 0 and GAMMA:
                    g_km1 = float(gam[k - 1])
                    if need_ys:
                        # ys = gamma_{k-1} * y (ACT, no deps on this iter)
                        for m in range(2):
                            nc.scalar.activation(
                                ys2[m][:],
                                y2[m][:].bitcast(F32),
                                ACTF.Copy,
                                bias=0.0,
                                scale=g_km1,
                            )
                    npr = YSUB_GRAN // 512
                    for nb in range(4 // npr):
                        bsl = slice(nb * YSUB_GRAN, (nb + 1) * YSUB_GRAN)
                        for m in range(2):
                            pr = prp.tile([128, YSUB_GRAN], F32, tag="pr", name="pr")
                            for ns in range(npr):
                                n = nb * npr + ns
                                nsl = slice(n * 512, (n + 1) * 512)
                                psl = slice(ns * 512, (ns + 1) * 512)
                                for kx in range(4):
                                    nc.tensor.matmul(
                                        pr[:, psl],
                                        DTt[:, kx * 256 + m * 128 : kx * 256 + (m + 1) * 128],
                                        z32[kx][:, nsl],
                                        start=(kx == 0),
                                        stop=(kx == 3),
                                    )
                            if YSUB_ENG[nb * 2 + m] == "v":
                                nc.vector.scalar_tensor_tensor(
                                    r2[m][:, bsl],
                                    y2[m][:, bsl].bitcast(F32),
                                    g_km1,
                                    pr[:],
                                    op0=ALU.mult,
                                    op1=ALU.add,
                                )
                            else:
                                # r = psum (ACT), then r += ys (Pool, SBUF)
                                nc.scalar.activation(
                                    r2[m][:, bsl], pr[:], ACTF.Copy
                                )
                                nc.gpsimd.tensor_tensor(
                                    r2[m][:, bsl],
                                    r2[m][:, bsl].bitcast(F32),
                                    ys2[m][:, bsl],
                                    op=ALU.add,
                                )
                elif k > 0:
                    for n in range(4):
                        nsl = slice(n * 512, (n + 1) * 512)
                        for m in range(2):
                            pr = prp.tile([128, 512], F32, tag="pr", name="pr")
                            for kx in range(4):
                                nc.tensor.matmul(
                                    pr[:],
                                    DTt[:, kx * 256 + m * 128 : kx * 256 + (m + 1) * 128],
                                    z32[kx][:, nsl],
                                    start=(kx == 0),
                                    stop=(kx == 3),
                                )
                            if YSUB_ENG[n * 2 + m] == "v":
                                nc.vector.scalar_tensor_tensor(
                                    r2[m][:, nsl],
                                    y2[m][:, nsl].bitcast(F32),
                                    -1.0,
                                    pr[:],
                                    op0=ALU.mult,
                                    op1=ALU.add,
                                )
                            else:
                                nc.scalar.activation(
                                    r2[m][:, nsl], pr[:], ACTF.Copy
                                )
                                nc.gpsimd.tensor_tensor(
                                    r2[m][:, nsl],
                                    r2[m][:, nsl].bitcast(F32),
                                    y2[m][:, nsl].bitcast(F32),
                                    op=ALU.subtract,
                                )

                # ---- phase B: psum_u = (step*D)^T @ (r | y); shrink; momentum
                for nb in range(n_pb):
                    hsl = slice(nb * PB_GRAN, (nb + 1) * PB_GRAN)
                    for mx in range(4):
                        ci = nb * 4 + mx
                        pu = pup.tile([128, PB_GRAN], F32, tag="pu", name="pu")
                        for ns in range(PB_GRAN // 512):
                            nsl = slice(
                                nb * PB_GRAN + ns * 512,
                                nb * PB_GRAN + (ns + 1) * 512,
                            )
                            psl = slice(ns * 512, (ns + 1) * 512)
                            mov = y2 if k == 0 else r2
                            for kc in range(2):
                                nc.tensor.matmul(
                                    pu[:, psl],
                                    sDt[:, kc * 512 + mx * 128 : kc * 512 + (mx + 1) * 128],
                                    mov[kc][:, nsl],
                                    start=(kc == 0),
                                    stop=(kc == 1),
                                )
                        if GAMMA:
                            # O'_k = gamma_k O_k: C0 and the clip absorb the
                            # rescale (k=0: psum = +b, in1 = zeros)
                            if k == 0:
                                s0 = float(-gam[0] * cs[0])
                                cl = float(abs(gam[0]) * cthrs[0])
                            else:
                                s0 = float(-cs[k] * gam[k] / gam[k - 1])
                                cl = float(abs(gam[k]) * cthrs[k])
                        else:
                            s0 = c_k if k > 0 else -c_k
                            cl = cthr_k
                        nc.vector._custom_dve(
                            FSHRINK,
                            out=O_new[mx][:, hsl],
                            in0=pu[:],
                            in1=(
                                z32[mx][:, hsl].bitcast(F32)
                                if k > 0
                                else (zb[:] if GAMMA else O_old[mx][:, hsl])
                            ),
                            s0=s0,
                            s1=-cl,
                            imm2=cl,
                        )
                        # momentum
                        if GAMMA:
                            # Z_k = O'_k - O'_{k-1}; at k=0 simply Z_0 = O'_0
                            if k == 0:
                                nc.gpsimd.tensor_copy(
                                    z32[mx][:, hsl], O_new[mx][:, hsl]
                                )
                            elif MOM_ENG[ci] == "v":
                                nc.vector.scalar_tensor_tensor(
                                    z32[mx][:, hsl],
                                    O_old[mx][:, hsl],
                                    -1.0,
                                    O_new[mx][:, hsl],
                                    op0=ALU.mult,
                                    op1=ALU.add,
                                )
                            else:
                                nc.gpsimd.tensor_tensor(
                                    z32[mx][:, hsl],
                                    O_new[mx][:, hsl],
                                    O_old[mx][:, hsl],
                                    op=ALU.subtract,
                                )
                        elif k == 0:
                            # beta_0 = 0 and O_old is zeros: z = -O_new
                            nc.gpsimd.tensor_tensor(
                                z32[mx][:, hsl],
                                O_old[mx][:, hsl],
                                O_new[mx][:, hsl],
                                op=ALU.subtract,
                            )
                        elif MOM_ENG[ci] == "v":
                            nc.vector.scalar_tensor_tensor(
                                z32[mx][:, hsl],
                                O_old[mx][:, hsl],
                                beta_k,
                                O_new[mx][:, hsl],
                                op0=ALU.mult,
                                op1=ALU.subtract,
                            )
                        else:
                            if MOM_ENG[ci] == "ap":
                                nc.scalar.activation(
                                    O_old[mx][:, hsl],
                                    O_old[mx][:, hsl],
                                    ACTF.Copy,
                                    bias=0.0,
                                    scale=beta_k,
                                )
                            else:  # "gp"
                                nc.gpsimd.tensor_scalar_mul(
                                    O_old[mx][:, hsl], O_old[mx][:, hsl], beta_k
                                )
                            nc.gpsimd.tensor_tensor(
                                z32[mx][:, hsl],
                                O_old[mx][:, hsl],
                                O_new[mx][:, hsl],
                                op=ALU.subtract,
                            )
                        if k == n_iter - 1:
                            # stream the output as each chunk finalizes
                            eng = (nc.sync, nc.scalar, nc.gpsimd, nc.sync)[mx]
                            eng.dma_start(
                                out_d[mx * 128 : (mx + 1) * 128, hsl],
                                z32[mx][:, hsl].bitcast(F32),
                            )

    nc.compile()
    _BUILD_CACHE[key] = nc
    return nc


# ------------------------------------------------------- host-side driver
def _host_inputs(y, D):
    DT = np.ascontiguousarray(D.T.astype(np.float32))
    sD = np.ascontiguousarray((np.float32(1.0 / LIPSCHITZ) * D).astype(np.float32))
    in_maps = []
    for c in range(N_CORES):
        ysh = np.ascontiguousarray(y[:, c * NSH : (c + 1) * NSH].astype(np.float32))
        in_maps.append({"y": ysh, "dt": DT, "sd": sD})
    return in_maps


LAST_EXEC_NS = None


def kernel(y, D):
    global LAST_EXEC_NS
    import os

    from concourse.bass_utils import run_bass_kernel_spmd

    y = np.asarray(y, dtype=np.float32)
    D = np.asarray(D, dtype=np.float32)
    assert y.shape == (DIM_Y, N_SAMPLES) and D.shape == (DIM_Y, DIM_X)

    nc = _build(N_ITER)
    in_maps = _host_inputs(y, D)
    trace = os.environ.get("DEEPISTA_TRACE", "0") == "1"
    r = run_bass_kernel_spmd(nc, in_maps, list(range(N_CORES)), trace=trace)
    LAST_EXEC_NS = r.exec_time_ns
    out = np.concatenate([r.results[c]["out"] for c in range(N_CORES)], axis=1)
    return out.astype(np.float32)
